# revision 17
# baseline (speedup 1.0000x reference)
"""Trainium2 Bass kernel for nn_DistMax (v3: banded scheme).

Reference semantics (n=1024, d=256):
    pair_max[i,j,:] = max(xs[i], xs[j])
    d_pred[i,j]     = dot(pair_max[i,j], w) + b
    mask[i,j]       = (batch[i] == batch[j]) && (i != j)
    out1            = relu(d_pred * mask);  count = sum(mask)
returns (out1, mask, count) like the reference.

Math: max(a,b) = a + relu(b-a)  =>
    d_pred[i,j] = s_i + b + sum_d w_d * relu(x_jd - x_id),  s = xs @ w.

Key observation: batch is SORTED, so mask (and therefore out1, since
out1 = mask * relu(d_pred)) is nonzero only within a diagonal band of
halfwidth (max group size - 1).  Everything outside the band is exactly
0 in the reference as well.  So each core (128 rows) computes only a
wrapped column window [r0 - pad, r0 + 128 + pad) of width W = 128+2*pad,
with W picked at runtime from the actual group sizes (256 covers group
spans <= 65; 384/512/...; 1152 covers any batch since the window then
wraps the full 1024 columns).

Per row q (psum partition p = 32*(q%4) + q//4; strips rotate so each
LDWEIGHTS targets a different PE column group than the running matmul):
  - relu tile R_c = relu(xsT_c - x_i) [128d x Wj] fp16 on ScalarE
    (activation(Relu, bias=-x_i)) or VectorE (tensor_scalar(sub,max,0)),
    split 1:3 to balance engine throughput.
  - TensorE accumulates row i into PSUM partition p with stationary
    w-selector [128,32] (sliding slice of a [128,63] [0..0 w 0..0]
    buffer) via col-group tile_position; N chunks of <=512 (bank-aligned).
Finalize (per 32-row strip, pipelined): relu(PSUM + (s_i+b)) on ScalarE,
multiply by mask on VectorE, DMA out.
mask = is_equal(batch_row, batch_col) in one VectorE op; the host adds
0.5 to the diagonal element of the replicated batch row so i==j fails
the equality (removes the eye without an extra op or input).
count = sum_g n_g^2 - n (exact in f32) from host bincount.
"""

import sys
import time

import numpy as np

if "/opt/trn_rl_repo" not in sys.path:
    sys.path.insert(0, "/opt/trn_rl_repo")

N = 1024
D = 256
NCORES = 8
RPC = N // NCORES  # 128 rows per core

_CACHE = {}

# loop index q -> psum partition p = 32*(q%4) + q//4: consecutive rows
# rotate across the four PE column-group strips, so each LDWEIGHTS
# targets a different col-group than the running matmul and overlaps it.
_P_OF_Q = [32 * (q % 4) + q // 4 for q in range(RPC)]

_ALLOWED_W = [192, 224, 256, 320, 384, 512, 640, 768, 896, 1024, 1152]


def _nchunks(W):
    """Bank-aligned matmul N chunks covering [0, W)."""
    chunks = []
    off = 0
    while off < W:
        n = min(512, W - off)
        chunks.append((off, n))
        off += n
    return chunks


def _build(reps: int, W: int):
    import contextlib

    import concourse.bass as bass  # noqa: F401
    import concourse.mybir as mybir
    from concourse import bacc, tile

    f32 = mybir.dt.float32
    f16 = mybir.dt.float16

    nc = bacc.Bacc("TRN2", target_bir_lowering=False, debug=False,
                   num_devices=NCORES)

    xt_d = nc.dram_tensor("xt", [2, 128, W], f16, kind="ExternalInput")
    xc_d = nc.dram_tensor("xc", [2, 128, RPC], f32, kind="ExternalInput")
    nx_d = nc.dram_tensor("nx", [2, 128, RPC], f32, kind="ExternalInput")
    w63_d = nc.dram_tensor("w63", [2, 128, 63], f16, kind="ExternalInput")
    sib_d = nc.dram_tensor("sib", [128, 1], f32, kind="ExternalInput")
    brow_d = nc.dram_tensor("brow", [128, W], f32, kind="ExternalInput")
    bcol_d = nc.dram_tensor("bcol", [128, 1], f32, kind="ExternalInput")
    o1_d = nc.dram_tensor("o1", [128, W], f32, kind="ExternalOutput")
    mk_d = nc.dram_tensor("mk", [128, W], f32, kind="ExternalOutput")

    with tile.TileContext(nc) as tc:
        with (
            tc.tile_pool(name="const", bufs=1) as cpool,
            tc.tile_pool(name="rhs", bufs=10) as rpool,
            tc.tile_pool(name="out", bufs=1) as opool,
            tc.tile_pool(name="psum", bufs=1, space="PSUM") as ppool,
        ):
            xt = [cpool.tile([128, W], f16, tag=f"xt{c}", name=f"xt{c}")
                  for c in range(2)]
            xc = [cpool.tile([128, RPC], f32, tag=f"xc{c}", name=f"xc{c}")
                  for c in range(2)]
            nx = [cpool.tile([128, RPC], f32, tag=f"nx{c}", name=f"nx{c}")
                  for c in range(2)]
            w63 = [cpool.tile([128, 63], f16, tag=f"w63{c}", name=f"w63{c}")
                   for c in range(2)]
            sib = cpool.tile([128, 1], f32, tag="sib", name="sib")
            brow = cpool.tile([128, W], f32, tag="brow", name="brow")
            bcol = cpool.tile([128, 1], f32, tag="bcol", name="bcol")

            for c in range(2):
                nc.sync.dma_start(out=xt[c][:], in_=xt_d[c])
                nc.sync.dma_start(out=xc[c][:], in_=xc_d[c])
                nc.sync.dma_start(out=nx[c][:], in_=nx_d[c])
                nc.sync.dma_start(out=w63[c][:], in_=w63_d[c])
            nc.sync.dma_start(out=sib[:], in_=sib_d[:])
            nc.sync.dma_start(out=brow[:], in_=brow_d[:])
            nc.sync.dma_start(out=bcol[:], in_=bcol_d[:])

            chunks = _nchunks(W)
            loop_cm = (tc.For_i(0, reps, 1) if reps > 1
                       else contextlib.nullcontext())
            with loop_cm:
                # mask = (b_i == b_j); diagonal removed via poisoned brow
                mksb = opool.tile([128, W], f32, tag="mksb", name="mksb")
                nc.vector.tensor_scalar(
                    out=mksb[:], in0=brow[:], scalar1=bcol[:, 0:1],
                    scalar2=None, op0=mybir.AluOpType.is_equal)
                nc.sync.dma_start(out=mk_d[:], in_=mksb[:])

                ps = ppool.tile([128, W], f32, tag="ps", name="ps")
                for q in range(RPC):
                    s, m = q % 4, q // 4
                    on_act = (q % 4 == 3)
                    for c in range(2):
                        rt = rpool.tile([128, W], f16, tag="rt", name="rt")
                        if on_act:
                            nc.scalar.activation(
                                out=rt[:], in_=xt[c][:],
                                func=mybir.ActivationFunctionType.Relu,
                                bias=nx[c][:, q:q + 1], scale=1.0)
                        else:
                            nc.vector.tensor_scalar(
                                out=rt[:], in0=xt[c][:],
                                scalar1=xc[c][:, q:q + 1], scalar2=0.0,
                                op0=mybir.AluOpType.subtract,
                                op1=mybir.AluOpType.max)
                        for off, nn_ in chunks:
                            nc.tensor.matmul(
                                ps[32 * s:32 * s + 32, off:off + nn_],
                                w63[c][:, 31 - m:63 - m],
                                rt[:, off:off + nn_],
                                start=(m == 0 and c == 0),
                                stop=(m == 31 and c == 1),
                                tile_position=(0, 32 * s))

                relu = opool.tile([128, W], f32, tag="relu", name="relu")
                o1sb = opool.tile([128, W], f32, tag="o1sb", name="o1sb")
                for s in range(4):
                    r = slice(32 * s, 32 * s + 32)
                    nc.scalar.activation(
                        out=relu[r, :], in_=ps[r, :],
                        func=mybir.ActivationFunctionType.Relu,
                        bias=sib[r, 0:1], scale=1.0)
                    nc.vector.tensor_tensor(
                        out=o1sb[r, :], in0=relu[r, :], in1=mksb[r, :],
                        op=mybir.AluOpType.mult)
                    nc.sync.dma_start(out=o1_d[r, :], in_=o1sb[r, :])

    nc.compile()
    return nc


def _pick_W(batch):
    b = np.asarray(batch).astype(np.int64)
    counts = np.bincount(b, minlength=1)
    span = int(counts.max())  # max group size; need pad >= span - 1
    for Wc in _ALLOWED_W:
        pad = (Wc - RPC) // 2
        if Wc >= N + RPC or pad >= span - 1:
            return Wc
    return _ALLOWED_W[-1]


def _prep_inputs(xs, batch, w, b, W):
    """Host-side slicing/packing/rolling. All O(n*d) numpy."""
    xs = np.ascontiguousarray(xs, dtype=np.float32)
    batch_f = np.ascontiguousarray(batch, dtype=np.float32)
    w = np.asarray(w, dtype=np.float32)
    bval = np.float32(np.asarray(b).reshape(-1)[0])
    pad = (W - RPC) // 2

    xsT = xs.T  # [256, 1024]
    xsT16 = xsT.astype(np.float16)
    s = (xs @ w + bval).astype(np.float32)  # s_i + b

    w63 = np.zeros((2, 128, 63), np.float16)
    w63[0, :, 31] = w[:128].astype(np.float16)
    w63[1, :, 31] = w[128:].astype(np.float16)

    perm = np.array(_P_OF_Q)  # producer col q -> local row perm[q]

    in_maps = []
    for cid in range(NCORES):
        r0 = cid * RPC
        cols = (r0 - pad + np.arange(W)) % N
        xblk = xsT[:, r0:r0 + RPC]          # [256, 128] f32, local rows
        browc = np.ascontiguousarray(
            np.broadcast_to(batch_f[cols], (128, W)).copy())
        # poison the diagonal so is_equal gives 0 there (mask removes i==j)
        browc[np.arange(128), np.arange(128) + pad] += 0.5
        xp = xblk[:, perm]                   # producer order
        in_maps.append({
            "xt": np.ascontiguousarray(xsT16[:, cols].reshape(2, 128, W)),
            "xc": np.ascontiguousarray(xp.reshape(2, 128, RPC)),
            "nx": np.ascontiguousarray((-xp).reshape(2, 128, RPC)),
            "w63": w63,
            "sib": np.ascontiguousarray(s[r0:r0 + RPC, None]),
            "brow": browc,
            "bcol": np.ascontiguousarray(batch_f[r0:r0 + RPC, None]),
        })
    return in_maps


def _get_runner(reps: int, W: int):
    key = ("runner", reps, W)
    if key in _CACHE:
        return _CACHE[key]

    import jax
    import concourse.mybir as mybir
    from concourse.bass2jax import (_bass_exec_p, install_neuronx_cc_hook,
                                    partition_id_tensor)
    from jax.experimental.shard_map import shard_map
    from jax.sharding import Mesh, NamedSharding, PartitionSpec

    nc = _build(reps, W)
    install_neuronx_cc_hook()

    partition_name = (nc.partition_id_tensor.name
                      if nc.partition_id_tensor else None)
    in_names, out_names, out_avals = [], [], []
    for alloc in nc.m.functions[0].allocations:
        if not isinstance(alloc, mybir.MemoryLocationSet):
            continue
        name = alloc.memorylocations[0].name
        if alloc.kind == "ExternalInput":
            if name != partition_name:
                in_names.append(name)
        elif alloc.kind == "ExternalOutput":
            out_names.append(name)
            out_avals.append(jax.core.ShapedArray(
                tuple(alloc.tensor_shape), mybir.dt.np(alloc.dtype)))
    n_params = len(in_names)
    n_outs = len(out_avals)
    in_names_all = in_names + out_names + (
        [partition_name] if partition_name else [])

    def _body(*args):
        operands = list(args)
        if partition_name is not None:
            operands.append(partition_id_tensor())
        return tuple(_bass_exec_p.bind(
            *operands, out_avals=tuple(out_avals),
            in_names=tuple(in_names_all), out_names=tuple(out_names),
            lowering_input_output_aliases=(), sim_require_finite=True,
            sim_require_nnan=True, nc=nc))

    devices = jax.devices()[:NCORES]
    mesh = Mesh(np.asarray(devices), ("core",))
    sharded = jax.jit(shard_map(
        _body, mesh=mesh,
        in_specs=(PartitionSpec("core"),) * (n_params + n_outs),
        out_specs=(PartitionSpec("core"),) * n_outs, check_rep=False))
    sharding = NamedSharding(mesh, PartitionSpec("core"))
    zeros_dev = [jax.device_put(
        np.zeros((NCORES * av.shape[0], *av.shape[1:]), av.dtype), sharding)
        for av in out_avals]

    def upload(in_maps):
        concat_in = [np.concatenate([in_maps[c][n] for c in range(NCORES)],
                                    axis=0) for n in in_names]
        return [jax.device_put(a, sharding) for a in concat_in]

    def execute(dev_in):
        return sharded(*dev_in, *zeros_dev)

    def fetch(outs):
        res = {}
        for i, name in enumerate(out_names):
            arr = np.asarray(outs[i])
            res[name] = arr.reshape(NCORES, *out_avals[i].shape)
        return res

    def run(in_maps):
        return fetch(execute(upload(in_maps)))

    run.upload = upload
    run.execute = execute
    run.fetch = fetch
    _CACHE[key] = run
    return run


def _assemble(res, W):
    pad = (W - RPC) // 2
    out1 = np.zeros((N, N), np.float32)
    mask = np.zeros((N, N), np.float32)
    ar = np.arange(W)
    for cid in range(NCORES):
        r0 = cid * RPC
        cols = (r0 - pad + ar) % N
        out1[r0:r0 + RPC, cols] = res["o1"][cid]
        mask[r0:r0 + RPC, cols] = res["mk"][cid]
    return out1, mask


def kernel(xs, batch, w, b):
    W = _pick_W(batch)
    run = _get_runner(1, W)
    in_maps = _prep_inputs(xs, batch, w, b, W)
    res = run(in_maps)
    out1, mask = _assemble(res, W)
    counts = np.bincount(np.asarray(batch).astype(np.int64), minlength=1)
    count = np.float32((counts.astype(np.int64) ** 2).sum() - N)
    return out1, mask, count


if __name__ == "__main__":
    rng = np.random.default_rng(0)
    xs = rng.standard_normal((N, D), dtype=np.float32)
    batch = np.sort(rng.integers(0, 32, N)).astype(np.int64)
    w = (rng.standard_normal(D, dtype=np.float32) / 16.0)
    b = rng.standard_normal(1, dtype=np.float32)
    t0 = time.time()
    o1, mk, cnt = kernel(xs=xs, batch=batch, w=w, b=b)
    print("kernel ran in", time.time() - t0, "W =", _pick_W(batch))
    pm = np.maximum(xs[:, None, :], xs[None, :, :])
    dp = pm @ w + b[0]
    same = (batch[:, None] == batch[None, :]).astype(np.float32)
    m_ref = same * (1.0 - np.eye(N, dtype=np.float32))
    o_ref = np.maximum(dp * m_ref, 0.0)
    print("out1 err:", np.abs(o1 - o_ref).max(),
          "rel:", np.abs(o1 - o_ref).max() / np.abs(o_ref).max())
    print("mask err:", np.abs(mk - m_ref).max())
    print("count:", cnt, "ref:", m_ref.sum())


# revision 21
# speedup vs baseline: 1.0245x; 1.0245x over previous
"""Trainium2 Bass kernel for nn_DistMax (v3: banded scheme).

Reference semantics (n=1024, d=256):
    pair_max[i,j,:] = max(xs[i], xs[j])
    d_pred[i,j]     = dot(pair_max[i,j], w) + b
    mask[i,j]       = (batch[i] == batch[j]) && (i != j)
    out1            = relu(d_pred * mask);  count = sum(mask)
returns (out1, mask, count) like the reference.

Math: max(a,b) = a + relu(b-a)  =>
    d_pred[i,j] = s_i + b + sum_d w_d * relu(x_jd - x_id),  s = xs @ w.

Key observation: batch is SORTED, so mask (and therefore out1, since
out1 = mask * relu(d_pred)) is nonzero only within a diagonal band of
halfwidth (max group size - 1).  Everything outside the band is exactly
0 in the reference as well.  So each core (128 rows) computes only a
wrapped column window [r0 - pad, r0 + 128 + pad) of width W = 128+2*pad,
with W picked at runtime from the actual group sizes (256 covers group
spans <= 65; 384/512/...; 1152 covers any batch since the window then
wraps the full 1024 columns).

Per row q (psum partition p = 32*(q%4) + q//4; strips rotate so each
LDWEIGHTS targets a different PE column group than the running matmul):
  - relu tile R_c = relu(xsT_c - x_i) [128d x Wj] fp16 on ScalarE
    (activation(Relu, bias=-x_i)) or VectorE (tensor_scalar(sub,max,0)),
    split 1:3 to balance engine throughput.
  - TensorE accumulates row i into PSUM partition p with stationary
    w-selector [128,32] (sliding slice of a [128,63] [0..0 w 0..0]
    buffer) via col-group tile_position; N chunks of <=512 (bank-aligned).
Finalize (per 32-row strip, pipelined): relu(PSUM + (s_i+b)) on ScalarE,
multiply by mask on VectorE, DMA out.
mask = is_equal(batch_row, batch_col) in one VectorE op; the host adds
0.5 to the diagonal element of the replicated batch row so i==j fails
the equality (removes the eye without an extra op or input).
count = sum_g n_g^2 - n (exact in f32) from host bincount.
"""

import sys
import time

import numpy as np

if "/opt/trn_rl_repo" not in sys.path:
    sys.path.insert(0, "/opt/trn_rl_repo")

N = 1024
D = 256
NCORES = 8
RPC = N // NCORES  # 128 rows per core

_CACHE = {}

# loop index q -> psum partition p = 32*(q%4) + q//4: consecutive rows
# rotate across the four PE column-group strips, so each LDWEIGHTS
# targets a different col-group than the running matmul and overlaps it.
_P_OF_Q = [32 * (q % 4) + q // 4 for q in range(RPC)]

_ALLOWED_W = [192, 216, 224, 256, 320, 384, 512, 640, 768, 896, 1024, 1152]


def _nchunks(W):
    """Bank-aligned matmul N chunks covering [0, W)."""
    chunks = []
    off = 0
    while off < W:
        n = min(512, W - off)
        chunks.append((off, n))
        off += n
    return chunks


def _build(reps: int, W: int):
    import contextlib

    import concourse.bass as bass  # noqa: F401
    import concourse.mybir as mybir
    from concourse import bacc, tile

    f32 = mybir.dt.float32
    f16 = mybir.dt.float16

    nc = bacc.Bacc("TRN2", target_bir_lowering=False, debug=False,
                   num_devices=NCORES)

    xt_d = nc.dram_tensor("xt", [2, 128, W], f16, kind="ExternalInput")
    xc_d = nc.dram_tensor("xc", [2, 128, RPC], f32, kind="ExternalInput")
    nx_d = nc.dram_tensor("nx", [2, 128, RPC], f32, kind="ExternalInput")
    w63_d = nc.dram_tensor("w63", [2, 128, 63], f16, kind="ExternalInput")
    sib_d = nc.dram_tensor("sib", [128, 1], f32, kind="ExternalInput")
    brow_d = nc.dram_tensor("brow", [128, W], f32, kind="ExternalInput")
    bcol_d = nc.dram_tensor("bcol", [128, 1], f32, kind="ExternalInput")
    o1_d = nc.dram_tensor("o1", [128, W], f32, kind="ExternalOutput")
    mk_d = nc.dram_tensor("mk", [128, W], f32, kind="ExternalOutput")

    with tile.TileContext(nc) as tc:
        with (
            tc.tile_pool(name="const", bufs=1) as cpool,
            tc.tile_pool(name="rhs", bufs=10) as rpool,
            tc.tile_pool(name="out", bufs=1) as opool,
            tc.tile_pool(name="psum", bufs=1, space="PSUM") as ppool,
        ):
            xt = [cpool.tile([128, W], f16, tag=f"xt{c}", name=f"xt{c}")
                  for c in range(2)]
            xc = [cpool.tile([128, RPC], f32, tag=f"xc{c}", name=f"xc{c}")
                  for c in range(2)]
            nx = [cpool.tile([128, RPC], f32, tag=f"nx{c}", name=f"nx{c}")
                  for c in range(2)]
            w63 = [cpool.tile([128, 63], f16, tag=f"w63{c}", name=f"w63{c}")
                   for c in range(2)]
            sib = cpool.tile([128, 1], f32, tag="sib", name="sib")
            brow = cpool.tile([128, W], f32, tag="brow", name="brow")
            bcol = cpool.tile([128, 1], f32, tag="bcol", name="bcol")

            for c in range(2):
                nc.sync.dma_start(out=xt[c][:], in_=xt_d[c])
                nc.sync.dma_start(out=xc[c][:], in_=xc_d[c])
                nc.sync.dma_start(out=nx[c][:], in_=nx_d[c])
                nc.sync.dma_start(out=w63[c][:], in_=w63_d[c])
            nc.sync.dma_start(out=sib[:], in_=sib_d[:])
            nc.sync.dma_start(out=brow[:], in_=brow_d[:])
            nc.sync.dma_start(out=bcol[:], in_=bcol_d[:])

            chunks = _nchunks(W)
            loop_cm = (tc.For_i(0, reps, 1, staggered_reset=True) if reps > 1
                       else contextlib.nullcontext())
            with loop_cm:
                # mask = (b_i == b_j); diagonal removed via poisoned brow
                mksb = opool.tile([128, W], f32, tag="mksb", name="mksb")
                nc.vector.tensor_scalar(
                    out=mksb[:], in0=brow[:], scalar1=bcol[:, 0:1],
                    scalar2=None, op0=mybir.AluOpType.is_equal)
                nc.sync.dma_start(out=mk_d[:], in_=mksb[:])

                ps = ppool.tile([128, W], f32, tag="ps", name="ps")
                for q in range(RPC):
                    s, m = q % 4, q // 4
                    on_act = (q % 4 == 3)
                    for c in range(2):
                        rt = rpool.tile([128, W], f16, tag="rt", name="rt")
                        if on_act:
                            nc.scalar.activation(
                                out=rt[:], in_=xt[c][:],
                                func=mybir.ActivationFunctionType.Relu,
                                bias=nx[c][:, q:q + 1], scale=1.0)
                        else:
                            nc.vector.tensor_scalar(
                                out=rt[:], in0=xt[c][:],
                                scalar1=xc[c][:, q:q + 1], scalar2=0.0,
                                op0=mybir.AluOpType.subtract,
                                op1=mybir.AluOpType.max)
                        for off, nn_ in chunks:
                            nc.tensor.matmul(
                                ps[32 * s:32 * s + 32, off:off + nn_],
                                w63[c][:, 31 - m:63 - m],
                                rt[:, off:off + nn_],
                                start=(m == 0 and c == 0),
                                stop=(m == 31 and c == 1),
                                tile_position=(0, 32 * s))

                relu = opool.tile([128, W], f32, tag="relu", name="relu")
                o1sb = opool.tile([128, W], f32, tag="o1sb", name="o1sb")
                for s in range(4):
                    r = slice(32 * s, 32 * s + 32)
                    nc.scalar.activation(
                        out=relu[r, :], in_=ps[r, :],
                        func=mybir.ActivationFunctionType.Relu,
                        bias=sib[r, 0:1], scale=1.0)
                    nc.vector.tensor_tensor(
                        out=o1sb[r, :], in0=relu[r, :], in1=mksb[r, :],
                        op=mybir.AluOpType.mult)
                    nc.sync.dma_start(out=o1_d[r, :], in_=o1sb[r, :])

    nc.compile()
    return nc


def _pick_W(batch):
    b = np.asarray(batch).astype(np.int64)
    counts = np.bincount(b, minlength=1)
    span = int(counts.max())  # max group size; need pad >= span - 1
    for Wc in _ALLOWED_W:
        pad = (Wc - RPC) // 2
        if Wc >= N + RPC or pad >= span - 1:
            return Wc
    return _ALLOWED_W[-1]


def _prep_inputs(xs, batch, w, b, W):
    """Host-side slicing/packing/rolling. All O(n*d) numpy."""
    xs = np.ascontiguousarray(xs, dtype=np.float32)
    batch_f = np.ascontiguousarray(batch, dtype=np.float32)
    w = np.asarray(w, dtype=np.float32)
    bval = np.float32(np.asarray(b).reshape(-1)[0])
    pad = (W - RPC) // 2

    xsT = xs.T  # [256, 1024]
    xsT16 = xsT.astype(np.float16)
    s = (xs @ w + bval).astype(np.float32)  # s_i + b

    w63 = np.zeros((2, 128, 63), np.float16)
    w63[0, :, 31] = w[:128].astype(np.float16)
    w63[1, :, 31] = w[128:].astype(np.float16)

    perm = np.array(_P_OF_Q)  # producer col q -> local row perm[q]

    in_maps = []
    for cid in range(NCORES):
        r0 = cid * RPC
        cols = (r0 - pad + np.arange(W)) % N
        xblk = xsT[:, r0:r0 + RPC]          # [256, 128] f32, local rows
        browc = np.ascontiguousarray(
            np.broadcast_to(batch_f[cols], (128, W)).copy())
        # poison the diagonal so is_equal gives 0 there (mask removes i==j)
        browc[np.arange(128), np.arange(128) + pad] += 0.5
        xp = xblk[:, perm]                   # producer order
        in_maps.append({
            "xt": np.ascontiguousarray(xsT16[:, cols].reshape(2, 128, W)),
            "xc": np.ascontiguousarray(xp.reshape(2, 128, RPC)),
            "nx": np.ascontiguousarray((-xp).reshape(2, 128, RPC)),
            "w63": w63,
            "sib": np.ascontiguousarray(s[r0:r0 + RPC, None]),
            "brow": browc,
            "bcol": np.ascontiguousarray(batch_f[r0:r0 + RPC, None]),
        })
    return in_maps


def _get_runner(reps: int, W: int):
    key = ("runner", reps, W)
    if key in _CACHE:
        return _CACHE[key]

    import jax
    import concourse.mybir as mybir
    from concourse.bass2jax import (_bass_exec_p, install_neuronx_cc_hook,
                                    partition_id_tensor)
    from jax.experimental.shard_map import shard_map
    from jax.sharding import Mesh, NamedSharding, PartitionSpec

    nc = _build(reps, W)
    install_neuronx_cc_hook()

    partition_name = (nc.partition_id_tensor.name
                      if nc.partition_id_tensor else None)
    in_names, out_names, out_avals = [], [], []
    for alloc in nc.m.functions[0].allocations:
        if not isinstance(alloc, mybir.MemoryLocationSet):
            continue
        name = alloc.memorylocations[0].name
        if alloc.kind == "ExternalInput":
            if name != partition_name:
                in_names.append(name)
        elif alloc.kind == "ExternalOutput":
            out_names.append(name)
            out_avals.append(jax.core.ShapedArray(
                tuple(alloc.tensor_shape), mybir.dt.np(alloc.dtype)))
    n_params = len(in_names)
    n_outs = len(out_avals)
    in_names_all = in_names + out_names + (
        [partition_name] if partition_name else [])

    def _body(*args):
        operands = list(args)
        if partition_name is not None:
            operands.append(partition_id_tensor())
        return tuple(_bass_exec_p.bind(
            *operands, out_avals=tuple(out_avals),
            in_names=tuple(in_names_all), out_names=tuple(out_names),
            lowering_input_output_aliases=(), sim_require_finite=True,
            sim_require_nnan=True, nc=nc))

    devices = jax.devices()[:NCORES]
    mesh = Mesh(np.asarray(devices), ("core",))
    sharded = jax.jit(shard_map(
        _body, mesh=mesh,
        in_specs=(PartitionSpec("core"),) * (n_params + n_outs),
        out_specs=(PartitionSpec("core"),) * n_outs, check_rep=False))
    sharding = NamedSharding(mesh, PartitionSpec("core"))
    zeros_dev = [jax.device_put(
        np.zeros((NCORES * av.shape[0], *av.shape[1:]), av.dtype), sharding)
        for av in out_avals]

    def upload(in_maps):
        concat_in = [np.concatenate([in_maps[c][n] for c in range(NCORES)],
                                    axis=0) for n in in_names]
        return [jax.device_put(a, sharding) for a in concat_in]

    def execute(dev_in):
        return sharded(*dev_in, *zeros_dev)

    def fetch(outs):
        res = {}
        for i, name in enumerate(out_names):
            arr = np.asarray(outs[i])
            res[name] = arr.reshape(NCORES, *out_avals[i].shape)
        return res

    def run(in_maps):
        return fetch(execute(upload(in_maps)))

    run.upload = upload
    run.execute = execute
    run.fetch = fetch
    _CACHE[key] = run
    return run


def _assemble(res, W):
    pad = (W - RPC) // 2
    out1 = np.zeros((N, N), np.float32)
    mask = np.zeros((N, N), np.float32)
    ar = np.arange(W)
    for cid in range(NCORES):
        r0 = cid * RPC
        cols = (r0 - pad + ar) % N
        out1[r0:r0 + RPC, cols] = res["o1"][cid]
        mask[r0:r0 + RPC, cols] = res["mk"][cid]
    return out1, mask


def kernel(xs, batch, w, b):
    W = _pick_W(batch)
    run = _get_runner(1, W)
    in_maps = _prep_inputs(xs, batch, w, b, W)
    res = run(in_maps)
    out1, mask = _assemble(res, W)
    counts = np.bincount(np.asarray(batch).astype(np.int64), minlength=1)
    count = np.float32((counts.astype(np.int64) ** 2).sum() - N)
    return out1, mask, count


if __name__ == "__main__":
    rng = np.random.default_rng(0)
    xs = rng.standard_normal((N, D), dtype=np.float32)
    batch = np.sort(rng.integers(0, 32, N)).astype(np.int64)
    w = (rng.standard_normal(D, dtype=np.float32) / 16.0)
    b = rng.standard_normal(1, dtype=np.float32)
    t0 = time.time()
    o1, mk, cnt = kernel(xs=xs, batch=batch, w=w, b=b)
    print("kernel ran in", time.time() - t0, "W =", _pick_W(batch))
    pm = np.maximum(xs[:, None, :], xs[None, :, :])
    dp = pm @ w + b[0]
    same = (batch[:, None] == batch[None, :]).astype(np.float32)
    m_ref = same * (1.0 - np.eye(N, dtype=np.float32))
    o_ref = np.maximum(dp * m_ref, 0.0)
    print("out1 err:", np.abs(o1 - o_ref).max(),
          "rel:", np.abs(o1 - o_ref).max() / np.abs(o_ref).max())
    print("mask err:", np.abs(mk - m_ref).max())
    print("count:", cnt, "ref:", m_ref.sum())


# revision 23
# speedup vs baseline: 1.2563x; 1.2263x over previous
"""Trainium2 Bass kernel for nn_DistMax (v3: banded scheme).

Reference semantics (n=1024, d=256):
    pair_max[i,j,:] = max(xs[i], xs[j])
    d_pred[i,j]     = dot(pair_max[i,j], w) + b
    mask[i,j]       = (batch[i] == batch[j]) && (i != j)
    out1            = relu(d_pred * mask);  count = sum(mask)
returns (out1, mask, count) like the reference.

Math: max(a,b) = a + relu(b-a)  =>
    d_pred[i,j] = s_i + b + sum_d w_d * relu(x_jd - x_id),  s = xs @ w.

Key observation: batch is SORTED, so mask (and therefore out1, since
out1 = mask * relu(d_pred)) is nonzero only within a diagonal band of
halfwidth (max group size - 1).  Everything outside the band is exactly
0 in the reference as well.  So each core (128 rows) computes only a
wrapped column window [r0 - pad, r0 + 128 + pad) of width W = 128+2*pad,
with W picked at runtime from the actual group sizes (256 covers group
spans <= 65; 384/512/...; 1152 covers any batch since the window then
wraps the full 1024 columns).

Per row q (psum partition p = 32*(q%4) + q//4; strips rotate so each
LDWEIGHTS targets a different PE column group than the running matmul):
  - relu tile R_c = relu(xsT_c - x_i) [128d x Wj] fp16 on ScalarE
    (activation(Relu, bias=-x_i)) or VectorE (tensor_scalar(sub,max,0)),
    split 1:3 to balance engine throughput.
  - TensorE accumulates row i into PSUM partition p with stationary
    w-selector [128,32] (sliding slice of a [128,63] [0..0 w 0..0]
    buffer) via col-group tile_position; N chunks of <=512 (bank-aligned).
Finalize (per 32-row strip, pipelined): relu(PSUM + (s_i+b)) on ScalarE,
multiply by mask on VectorE, DMA out.
mask = is_equal(batch_row, batch_col) in one VectorE op; the host adds
0.5 to the diagonal element of the replicated batch row so i==j fails
the equality (removes the eye without an extra op or input).
count = sum_g n_g^2 - n (exact in f32) from host bincount.
"""

import sys
import time

import numpy as np

if "/opt/trn_rl_repo" not in sys.path:
    sys.path.insert(0, "/opt/trn_rl_repo")

N = 1024
D = 256
NCORES = 8
RPC = N // NCORES  # 128 rows per core

_CACHE = {}

# loop index q -> psum partition p = 32*(q%4) + q//4: consecutive rows
# rotate across the four PE column-group strips, so each LDWEIGHTS
# targets a different col-group than the running matmul and overlaps it.
_P_OF_Q = [32 * (q % 4) + q // 4 for q in range(RPC)]

_ALLOWED_W = [192, 216, 224, 256, 320, 384, 512, 640, 768, 896, 1024, 1152]


def _nchunks(W):
    """Bank-aligned matmul N chunks covering [0, W)."""
    chunks = []
    off = 0
    while off < W:
        n = min(512, W - off)
        chunks.append((off, n))
        off += n
    return chunks


def _build(reps: int, W: int):
    import contextlib

    import concourse.bass as bass  # noqa: F401
    import concourse.mybir as mybir
    from concourse import bacc, tile

    f32 = mybir.dt.float32
    f16 = mybir.dt.float16

    nc = bacc.Bacc("TRN2", target_bir_lowering=False, debug=False,
                   num_devices=NCORES)

    xt_d = nc.dram_tensor("xt", [2, 128, W], f16, kind="ExternalInput")
    xc_d = nc.dram_tensor("xc", [2, 128, RPC], f32, kind="ExternalInput")
    nx_d = nc.dram_tensor("nx", [2, 128, RPC], f32, kind="ExternalInput")
    w63_d = nc.dram_tensor("w63", [2, 128, 63], f16, kind="ExternalInput")
    sib_d = nc.dram_tensor("sib", [128, 1], f32, kind="ExternalInput")
    brow_d = nc.dram_tensor("brow", [128, W], f32, kind="ExternalInput")
    bcol_d = nc.dram_tensor("bcol", [128, 1], f32, kind="ExternalInput")
    o1_d = nc.dram_tensor("o1", [128, W], f32, kind="ExternalOutput")
    mk_d = nc.dram_tensor("mk", [128, W], f32, kind="ExternalOutput")

    with tile.TileContext(nc) as tc:
        with (
            tc.tile_pool(name="const", bufs=1) as cpool,
            tc.tile_pool(name="rhs", bufs=10) as rpool,
            tc.tile_pool(name="out", bufs=1) as opool,
            tc.tile_pool(name="psum", bufs=1, space="PSUM") as ppool,
        ):
            xt = [cpool.tile([128, W], f16, tag=f"xt{c}", name=f"xt{c}")
                  for c in range(2)]
            xc = [cpool.tile([128, RPC], f32, tag=f"xc{c}", name=f"xc{c}")
                  for c in range(2)]
            nx = [cpool.tile([128, RPC], f32, tag=f"nx{c}", name=f"nx{c}")
                  for c in range(2)]
            w63 = [cpool.tile([128, 63], f16, tag=f"w63{c}", name=f"w63{c}")
                   for c in range(2)]
            sib = cpool.tile([128, 1], f32, tag="sib", name="sib")
            brow = cpool.tile([128, W], f32, tag="brow", name="brow")
            bcol = cpool.tile([128, 1], f32, tag="bcol", name="bcol")

            for c in range(2):
                nc.sync.dma_start(out=xt[c][:], in_=xt_d[c])
                nc.sync.dma_start(out=xc[c][:], in_=xc_d[c])
                nc.sync.dma_start(out=nx[c][:], in_=nx_d[c])
                nc.sync.dma_start(out=w63[c][:], in_=w63_d[c])
            nc.sync.dma_start(out=sib[:], in_=sib_d[:])
            nc.sync.dma_start(out=brow[:], in_=brow_d[:])
            nc.sync.dma_start(out=bcol[:], in_=bcol_d[:])

            chunks = _nchunks(W)
            loop_cm = (tc.For_i(0, reps, 1, staggered_reset=True) if reps > 1
                       else contextlib.nullcontext())
            with loop_cm:
                # mask = (b_i == b_j); diagonal removed via poisoned brow
                mksb = opool.tile([128, W], f32, tag="mksb", name="mksb")
                nc.vector.tensor_scalar(
                    out=mksb[:], in0=brow[:], scalar1=bcol[:, 0:1],
                    scalar2=None, op0=mybir.AluOpType.is_equal)
                nc.sync.dma_start(out=mk_d[:], in_=mksb[:])

                ps = ppool.tile([128, W], f32, tag="ps", name="ps")
                for q in range(RPC):
                    s, m = q % 4, q // 4
                    on_act = (q % 4 == 3)
                    for c in range(2):
                        rt = rpool.tile([128, W], f16, tag="rt", name="rt")
                        if on_act:
                            nc.scalar.activation(
                                out=rt[:], in_=xt[c][:],
                                func=mybir.ActivationFunctionType.Relu,
                                bias=nx[c][:, q:q + 1], scale=1.0)
                        else:
                            nc.vector.tensor_scalar(
                                out=rt[:], in0=xt[c][:],
                                scalar1=xc[c][:, q:q + 1], scalar2=0.0,
                                op0=mybir.AluOpType.subtract,
                                op1=mybir.AluOpType.max)
                        for off, nn_ in chunks:
                            nc.tensor.matmul(
                                ps[32 * s:32 * s + 32, off:off + nn_],
                                w63[c][:, 31 - m:63 - m],
                                rt[:, off:off + nn_],
                                start=(m == 0 and c == 0),
                                stop=(m == 31 and c == 1),
                                tile_position=(0, 32 * s))

                relu = opool.tile([128, W], f32, tag="relu", name="relu")
                o1sb = opool.tile([128, W], f32, tag="o1sb", name="o1sb")
                for s in range(4):
                    r = slice(32 * s, 32 * s + 32)
                    nc.scalar.activation(
                        out=relu[r, :], in_=ps[r, :],
                        func=mybir.ActivationFunctionType.Relu,
                        bias=sib[r, 0:1], scale=1.0)
                    nc.vector.tensor_tensor(
                        out=o1sb[r, :], in0=relu[r, :], in1=mksb[r, :],
                        op=mybir.AluOpType.mult)
                    nc.sync.dma_start(out=o1_d[r, :], in_=o1sb[r, :])

    nc.compile()
    return nc



def _build_strip(reps: int, Ws: int):
    """Per-strip window variant: each 32-row PSUM strip s gets its own
    Ws-column window [32s - padS, 32s + 32 + padS), padS = (Ws-32)/2, so
    producer tiles and matmuls are [128, Ws] instead of the 128-row union
    window.  Valid when max group size <= padS + 1."""
    import contextlib

    import concourse.mybir as mybir
    from concourse import bacc, tile

    f32 = mybir.dt.float32
    f16 = mybir.dt.float16

    nc = bacc.Bacc("TRN2", target_bir_lowering=False, debug=False,
                   num_devices=NCORES)

    xt_d = nc.dram_tensor("xt", [2, 4, 128, Ws], f16, kind="ExternalInput")
    xc_d = nc.dram_tensor("xc", [2, 128, RPC], f32, kind="ExternalInput")
    nx_d = nc.dram_tensor("nx", [2, 128, RPC], f32, kind="ExternalInput")
    w63_d = nc.dram_tensor("w63", [2, 128, 63], f16, kind="ExternalInput")
    sib_d = nc.dram_tensor("sib", [128, 1], f32, kind="ExternalInput")
    brow_d = nc.dram_tensor("brow", [128, Ws], f32, kind="ExternalInput")
    bcol_d = nc.dram_tensor("bcol", [128, 1], f32, kind="ExternalInput")
    o1_d = nc.dram_tensor("o1", [128, Ws], f32, kind="ExternalOutput")
    mk_d = nc.dram_tensor("mk", [128, Ws], f32, kind="ExternalOutput")

    with tile.TileContext(nc) as tc:
        with (
            tc.tile_pool(name="const", bufs=1) as cpool,
            tc.tile_pool(name="rhs", bufs=16) as rpool,
            tc.tile_pool(name="out", bufs=1) as opool,
            tc.tile_pool(name="psum", bufs=1, space="PSUM") as ppool,
        ):
            xt = [[cpool.tile([128, Ws], f16, tag=f"xt{c}{st}",
                              name=f"xt{c}{st}") for st in range(4)]
                  for c in range(2)]
            xc = [cpool.tile([128, RPC], f32, tag=f"xc{c}", name=f"xc{c}")
                  for c in range(2)]
            nx = [cpool.tile([128, RPC], f32, tag=f"nx{c}", name=f"nx{c}")
                  for c in range(2)]
            w63 = [cpool.tile([128, 63], f16, tag=f"w63{c}", name=f"w63{c}")
                   for c in range(2)]
            sib = cpool.tile([128, 1], f32, tag="sib", name="sib")
            brow = cpool.tile([128, Ws], f32, tag="brow", name="brow")
            bcol = cpool.tile([128, 1], f32, tag="bcol", name="bcol")

            for c in range(2):
                for st in range(4):
                    nc.sync.dma_start(out=xt[c][st][:], in_=xt_d[c, st])
                nc.sync.dma_start(out=xc[c][:], in_=xc_d[c])
                nc.sync.dma_start(out=nx[c][:], in_=nx_d[c])
                nc.sync.dma_start(out=w63[c][:], in_=w63_d[c])
            nc.sync.dma_start(out=sib[:], in_=sib_d[:])
            nc.sync.dma_start(out=brow[:], in_=brow_d[:])
            nc.sync.dma_start(out=bcol[:], in_=bcol_d[:])

            loop_cm = (tc.For_i(0, reps, 1, staggered_reset=True)
                       if reps > 1 else contextlib.nullcontext())
            with loop_cm:
                mksb = opool.tile([128, Ws], f32, tag="mksb", name="mksb")
                nc.vector.tensor_scalar(
                    out=mksb[:], in0=brow[:], scalar1=bcol[:, 0:1],
                    scalar2=None, op0=mybir.AluOpType.is_equal)
                nc.sync.dma_start(out=mk_d[:], in_=mksb[:])

                ps = ppool.tile([128, Ws], f32, tag="ps", name="ps")
                for q in range(RPC):
                    s, m = q % 4, q // 4
                    on_act = (q % 3 == 2)
                    for c in range(2):
                        rt = rpool.tile([128, Ws], f16, tag="rt", name="rt")
                        if on_act:
                            nc.scalar.activation(
                                out=rt[:], in_=xt[c][s][:],
                                func=mybir.ActivationFunctionType.Relu,
                                bias=nx[c][:, q:q + 1], scale=1.0)
                        else:
                            nc.vector.tensor_scalar(
                                out=rt[:], in0=xt[c][s][:],
                                scalar1=xc[c][:, q:q + 1], scalar2=0.0,
                                op0=mybir.AluOpType.subtract,
                                op1=mybir.AluOpType.max)
                        for off, nn_ in _nchunks(Ws):
                            nc.tensor.matmul(
                                ps[32 * s:32 * s + 32, off:off + nn_],
                                w63[c][:, 31 - m:63 - m],
                                rt[:, off:off + nn_],
                                start=(m == 0 and c == 0),
                                stop=(m == 31 and c == 1),
                                tile_position=(0, 32 * s))

                relu = opool.tile([128, Ws], f32, tag="relu", name="relu")
                o1sb = opool.tile([128, Ws], f32, tag="o1sb", name="o1sb")
                nc.scalar.activation(
                    out=relu[:], in_=ps[:],
                    func=mybir.ActivationFunctionType.Relu,
                    bias=sib[:, 0:1], scale=1.0)
                nc.vector.tensor_tensor(
                    out=o1sb[:], in0=relu[:], in1=mksb[:],
                    op=mybir.AluOpType.mult)
                nc.sync.dma_start(out=o1_d[:], in_=o1sb[:])

    nc.compile()
    return nc


def _prep_inputs_strip(xs, batch, w, b, Ws):
    xs = np.ascontiguousarray(xs, dtype=np.float32)
    batch_f = np.ascontiguousarray(batch, dtype=np.float32)
    w = np.asarray(w, dtype=np.float32)
    bval = np.float32(np.asarray(b).reshape(-1)[0])
    padS = (Ws - 32) // 2

    xsT = xs.T
    xsT16 = xsT.astype(np.float16)
    s_vec = (xs @ w + bval).astype(np.float32)

    w63 = np.zeros((2, 128, 63), np.float16)
    w63[0, :, 31] = w[:128].astype(np.float16)
    w63[1, :, 31] = w[128:].astype(np.float16)

    perm = np.array(_P_OF_Q)
    ar = np.arange(Ws)

    in_maps = []
    for cid in range(NCORES):
        r0 = cid * RPC
        xblk = xsT[:, r0:r0 + RPC]
        xp = xblk[:, perm]
        # per-strip window columns and inputs
        xt = np.empty((2, 4, 128, Ws), np.float16)
        brow = np.empty((128, Ws), np.float32)
        for st in range(4):
            cols = (r0 + 32 * st - padS + ar) % N
            xt[:, st] = xsT16[:, cols].reshape(2, 128, Ws)
            brow[32 * st:32 * st + 32] = batch_f[cols]
        # poison diagonal: row p (strip p//32) has its own column at
        # local index (p % 32) + padS
        brow[np.arange(128), (np.arange(128) % 32) + padS] += 0.5
        in_maps.append({
            "xt": xt,
            "xc": np.ascontiguousarray(xp.reshape(2, 128, RPC)),
            "nx": np.ascontiguousarray((-xp).reshape(2, 128, RPC)),
            "w63": w63,
            "sib": np.ascontiguousarray(s_vec[r0:r0 + RPC, None]),
            "brow": brow,
            "bcol": np.ascontiguousarray(batch_f[r0:r0 + RPC, None]),
        })
    return in_maps


def _assemble_strip(res, Ws):
    padS = (Ws - 32) // 2
    out1 = np.zeros((N, N), np.float32)
    mask = np.zeros((N, N), np.float32)
    ar = np.arange(Ws)
    for cid in range(NCORES):
        r0 = cid * RPC
        for st in range(4):
            cols = (r0 + 32 * st - padS + ar) % N
            rr = slice(r0 + 32 * st, r0 + 32 * st + 32)
            out1[rr, cols] = res["o1"][cid][32 * st:32 * st + 32]
            mask[rr, cols] = res["mk"][cid][32 * st:32 * st + 32]
    return out1, mask


def _pick_W(batch):
    b = np.asarray(batch).astype(np.int64)
    counts = np.bincount(b, minlength=1)
    span = int(counts.max())  # max group size; need pad >= span - 1
    for padS in (48, 64, 80, 112):
        if span - 1 <= padS:
            return ("S", 32 + 2 * padS)
    for Wc in _ALLOWED_W:
        pad = (Wc - RPC) // 2
        if Wc >= N + RPC or pad >= span - 1:
            return Wc
    return _ALLOWED_W[-1]


def _prep_inputs(xs, batch, w, b, W):
    """Host-side slicing/packing/rolling. All O(n*d) numpy."""
    if isinstance(W, tuple):
        return _prep_inputs_strip(xs, batch, w, b, W[1])
    xs = np.ascontiguousarray(xs, dtype=np.float32)
    batch_f = np.ascontiguousarray(batch, dtype=np.float32)
    w = np.asarray(w, dtype=np.float32)
    bval = np.float32(np.asarray(b).reshape(-1)[0])
    pad = (W - RPC) // 2

    xsT = xs.T  # [256, 1024]
    xsT16 = xsT.astype(np.float16)
    s = (xs @ w + bval).astype(np.float32)  # s_i + b

    w63 = np.zeros((2, 128, 63), np.float16)
    w63[0, :, 31] = w[:128].astype(np.float16)
    w63[1, :, 31] = w[128:].astype(np.float16)

    perm = np.array(_P_OF_Q)  # producer col q -> local row perm[q]

    in_maps = []
    for cid in range(NCORES):
        r0 = cid * RPC
        cols = (r0 - pad + np.arange(W)) % N
        xblk = xsT[:, r0:r0 + RPC]          # [256, 128] f32, local rows
        browc = np.ascontiguousarray(
            np.broadcast_to(batch_f[cols], (128, W)).copy())
        # poison the diagonal so is_equal gives 0 there (mask removes i==j)
        browc[np.arange(128), np.arange(128) + pad] += 0.5
        xp = xblk[:, perm]                   # producer order
        in_maps.append({
            "xt": np.ascontiguousarray(xsT16[:, cols].reshape(2, 128, W)),
            "xc": np.ascontiguousarray(xp.reshape(2, 128, RPC)),
            "nx": np.ascontiguousarray((-xp).reshape(2, 128, RPC)),
            "w63": w63,
            "sib": np.ascontiguousarray(s[r0:r0 + RPC, None]),
            "brow": browc,
            "bcol": np.ascontiguousarray(batch_f[r0:r0 + RPC, None]),
        })
    return in_maps


def _get_runner(reps: int, W: int):
    key = ("runner", reps, W)
    if key in _CACHE:
        return _CACHE[key]

    import jax
    import concourse.mybir as mybir
    from concourse.bass2jax import (_bass_exec_p, install_neuronx_cc_hook,
                                    partition_id_tensor)
    from jax.experimental.shard_map import shard_map
    from jax.sharding import Mesh, NamedSharding, PartitionSpec

    nc = (_build_strip(reps, W[1]) if isinstance(W, tuple)
          else _build(reps, W))
    install_neuronx_cc_hook()

    partition_name = (nc.partition_id_tensor.name
                      if nc.partition_id_tensor else None)
    in_names, out_names, out_avals = [], [], []
    for alloc in nc.m.functions[0].allocations:
        if not isinstance(alloc, mybir.MemoryLocationSet):
            continue
        name = alloc.memorylocations[0].name
        if alloc.kind == "ExternalInput":
            if name != partition_name:
                in_names.append(name)
        elif alloc.kind == "ExternalOutput":
            out_names.append(name)
            out_avals.append(jax.core.ShapedArray(
                tuple(alloc.tensor_shape), mybir.dt.np(alloc.dtype)))
    n_params = len(in_names)
    n_outs = len(out_avals)
    in_names_all = in_names + out_names + (
        [partition_name] if partition_name else [])

    def _body(*args):
        operands = list(args)
        if partition_name is not None:
            operands.append(partition_id_tensor())
        return tuple(_bass_exec_p.bind(
            *operands, out_avals=tuple(out_avals),
            in_names=tuple(in_names_all), out_names=tuple(out_names),
            lowering_input_output_aliases=(), sim_require_finite=True,
            sim_require_nnan=True, nc=nc))

    devices = jax.devices()[:NCORES]
    mesh = Mesh(np.asarray(devices), ("core",))
    sharded = jax.jit(shard_map(
        _body, mesh=mesh,
        in_specs=(PartitionSpec("core"),) * (n_params + n_outs),
        out_specs=(PartitionSpec("core"),) * n_outs, check_rep=False))
    sharding = NamedSharding(mesh, PartitionSpec("core"))
    zeros_dev = [jax.device_put(
        np.zeros((NCORES * av.shape[0], *av.shape[1:]), av.dtype), sharding)
        for av in out_avals]

    def upload(in_maps):
        concat_in = [np.concatenate([in_maps[c][n] for c in range(NCORES)],
                                    axis=0) for n in in_names]
        return [jax.device_put(a, sharding) for a in concat_in]

    def execute(dev_in):
        return sharded(*dev_in, *zeros_dev)

    def fetch(outs):
        res = {}
        for i, name in enumerate(out_names):
            arr = np.asarray(outs[i])
            res[name] = arr.reshape(NCORES, *out_avals[i].shape)
        return res

    def run(in_maps):
        return fetch(execute(upload(in_maps)))

    run.upload = upload
    run.execute = execute
    run.fetch = fetch
    _CACHE[key] = run
    return run


def _assemble(res, W):
    if isinstance(W, tuple):
        return _assemble_strip(res, W[1])
    pad = (W - RPC) // 2
    out1 = np.zeros((N, N), np.float32)
    mask = np.zeros((N, N), np.float32)
    ar = np.arange(W)
    for cid in range(NCORES):
        r0 = cid * RPC
        cols = (r0 - pad + ar) % N
        out1[r0:r0 + RPC, cols] = res["o1"][cid]
        mask[r0:r0 + RPC, cols] = res["mk"][cid]
    return out1, mask


def kernel(xs, batch, w, b):
    W = _pick_W(batch)
    run = _get_runner(1, W)
    in_maps = _prep_inputs(xs, batch, w, b, W)
    res = run(in_maps)
    out1, mask = _assemble(res, W)
    counts = np.bincount(np.asarray(batch).astype(np.int64), minlength=1)
    count = np.float32((counts.astype(np.int64) ** 2).sum() - N)
    return out1, mask, count


if __name__ == "__main__":
    rng = np.random.default_rng(0)
    xs = rng.standard_normal((N, D), dtype=np.float32)
    batch = np.sort(rng.integers(0, 32, N)).astype(np.int64)
    w = (rng.standard_normal(D, dtype=np.float32) / 16.0)
    b = rng.standard_normal(1, dtype=np.float32)
    t0 = time.time()
    o1, mk, cnt = kernel(xs=xs, batch=batch, w=w, b=b)
    print("kernel ran in", time.time() - t0, "W =", _pick_W(batch))
    pm = np.maximum(xs[:, None, :], xs[None, :, :])
    dp = pm @ w + b[0]
    same = (batch[:, None] == batch[None, :]).astype(np.float32)
    m_ref = same * (1.0 - np.eye(N, dtype=np.float32))
    o_ref = np.maximum(dp * m_ref, 0.0)
    print("out1 err:", np.abs(o1 - o_ref).max(),
          "rel:", np.abs(o1 - o_ref).max() / np.abs(o_ref).max())
    print("mask err:", np.abs(mk - m_ref).max())
    print("count:", cnt, "ref:", m_ref.sum())


# revision 26
# speedup vs baseline: 1.3264x; 1.0558x over previous
"""Trainium2 Bass kernel for nn_DistMax (v3: banded scheme).

Reference semantics (n=1024, d=256):
    pair_max[i,j,:] = max(xs[i], xs[j])
    d_pred[i,j]     = dot(pair_max[i,j], w) + b
    mask[i,j]       = (batch[i] == batch[j]) && (i != j)
    out1            = relu(d_pred * mask);  count = sum(mask)
returns (out1, mask, count) like the reference.

Math: max(a,b) = a + relu(b-a)  =>
    d_pred[i,j] = s_i + b + sum_d w_d * relu(x_jd - x_id),  s = xs @ w.

Key observation: batch is SORTED, so mask (and therefore out1, since
out1 = mask * relu(d_pred)) is nonzero only within a diagonal band of
halfwidth (max group size - 1).  Everything outside the band is exactly
0 in the reference as well.  So each core (128 rows) computes only a
wrapped column window [r0 - pad, r0 + 128 + pad) of width W = 128+2*pad,
with W picked at runtime from the actual group sizes (256 covers group
spans <= 65; 384/512/...; 1152 covers any batch since the window then
wraps the full 1024 columns).

Per row q (psum partition p = 32*(q%4) + q//4; strips rotate so each
LDWEIGHTS targets a different PE column group than the running matmul):
  - relu tile R_c = relu(xsT_c - x_i) [128d x Wj] fp16 on ScalarE
    (activation(Relu, bias=-x_i)) or VectorE (tensor_scalar(sub,max,0)),
    split 1:3 to balance engine throughput.
  - TensorE accumulates row i into PSUM partition p with stationary
    w-selector [128,32] (sliding slice of a [128,63] [0..0 w 0..0]
    buffer) via col-group tile_position; N chunks of <=512 (bank-aligned).
Finalize (per 32-row strip, pipelined): relu(PSUM + (s_i+b)) on ScalarE,
multiply by mask on VectorE, DMA out.
mask = is_equal(batch_row, batch_col) in one VectorE op; the host adds
0.5 to the diagonal element of the replicated batch row so i==j fails
the equality (removes the eye without an extra op or input).
count = sum_g n_g^2 - n (exact in f32) from host bincount.
"""

import sys
import time

import numpy as np

if "/opt/trn_rl_repo" not in sys.path:
    sys.path.insert(0, "/opt/trn_rl_repo")

N = 1024
D = 256
NCORES = 8
RPC = N // NCORES  # 128 rows per core

_CACHE = {}

# loop index q -> psum partition p = 32*(q%4) + q//4: consecutive rows
# rotate across the four PE column-group strips, so each LDWEIGHTS
# targets a different col-group than the running matmul and overlaps it.
_P_OF_Q = [32 * (q % 4) + q // 4 for q in range(RPC)]

_ALLOWED_W = [192, 216, 224, 256, 320, 384, 512, 640, 768, 896, 1024, 1152]


def _nchunks(W):
    """Bank-aligned matmul N chunks covering [0, W)."""
    chunks = []
    off = 0
    while off < W:
        n = min(512, W - off)
        chunks.append((off, n))
        off += n
    return chunks


def _build(reps: int, W: int):
    import contextlib

    import concourse.bass as bass  # noqa: F401
    import concourse.mybir as mybir
    from concourse import bacc, tile

    f32 = mybir.dt.float32
    f16 = mybir.dt.float16

    nc = bacc.Bacc("TRN2", target_bir_lowering=False, debug=False,
                   num_devices=NCORES)

    xt_d = nc.dram_tensor("xt", [2, 128, W], f16, kind="ExternalInput")
    xc_d = nc.dram_tensor("xc", [2, 128, RPC], f32, kind="ExternalInput")
    nx_d = nc.dram_tensor("nx", [2, 128, RPC], f32, kind="ExternalInput")
    w63_d = nc.dram_tensor("w63", [2, 128, 63], f16, kind="ExternalInput")
    sib_d = nc.dram_tensor("sib", [128, 1], f32, kind="ExternalInput")
    brow_d = nc.dram_tensor("brow", [128, W], f32, kind="ExternalInput")
    bcol_d = nc.dram_tensor("bcol", [128, 1], f32, kind="ExternalInput")
    o1_d = nc.dram_tensor("o1", [128, W], f32, kind="ExternalOutput")
    mk_d = nc.dram_tensor("mk", [128, W], f32, kind="ExternalOutput")

    with tile.TileContext(nc) as tc:
        with (
            tc.tile_pool(name="const", bufs=1) as cpool,
            tc.tile_pool(name="rhs", bufs=10) as rpool,
            tc.tile_pool(name="out", bufs=1) as opool,
            tc.tile_pool(name="psum", bufs=1, space="PSUM") as ppool,
        ):
            xt = [cpool.tile([128, W], f16, tag=f"xt{c}", name=f"xt{c}")
                  for c in range(2)]
            xc = [cpool.tile([128, RPC], f32, tag=f"xc{c}", name=f"xc{c}")
                  for c in range(2)]
            nx = [cpool.tile([128, RPC], f32, tag=f"nx{c}", name=f"nx{c}")
                  for c in range(2)]
            w63 = [cpool.tile([128, 63], f16, tag=f"w63{c}", name=f"w63{c}")
                   for c in range(2)]
            sib = cpool.tile([128, 1], f32, tag="sib", name="sib")
            brow = cpool.tile([128, W], f32, tag="brow", name="brow")
            bcol = cpool.tile([128, 1], f32, tag="bcol", name="bcol")

            for c in range(2):
                nc.sync.dma_start(out=xt[c][:], in_=xt_d[c])
                nc.sync.dma_start(out=xc[c][:], in_=xc_d[c])
                nc.sync.dma_start(out=nx[c][:], in_=nx_d[c])
                nc.sync.dma_start(out=w63[c][:], in_=w63_d[c])
            nc.sync.dma_start(out=sib[:], in_=sib_d[:])
            nc.sync.dma_start(out=brow[:], in_=brow_d[:])
            nc.sync.dma_start(out=bcol[:], in_=bcol_d[:])

            chunks = _nchunks(W)
            loop_cm = (tc.For_i(0, reps, 1, staggered_reset=True) if reps > 1
                       else contextlib.nullcontext())
            with loop_cm:
                # mask = (b_i == b_j); diagonal removed via poisoned brow
                mksb = opool.tile([128, W], f32, tag="mksb", name="mksb")
                nc.vector.tensor_scalar(
                    out=mksb[:], in0=brow[:], scalar1=bcol[:, 0:1],
                    scalar2=None, op0=mybir.AluOpType.is_equal)
                nc.sync.dma_start(out=mk_d[:], in_=mksb[:])

                ps = ppool.tile([128, W], f32, tag="ps", name="ps")
                for q in range(RPC):
                    s, m = q % 4, q // 4
                    on_act = (q % 4 == 3)
                    for c in range(2):
                        rt = rpool.tile([128, W], f16, tag="rt", name="rt")
                        if on_act:
                            nc.scalar.activation(
                                out=rt[:], in_=xt[c][:],
                                func=mybir.ActivationFunctionType.Relu,
                                bias=nx[c][:, q:q + 1], scale=1.0)
                        else:
                            nc.vector.tensor_scalar(
                                out=rt[:], in0=xt[c][:],
                                scalar1=xc[c][:, q:q + 1], scalar2=0.0,
                                op0=mybir.AluOpType.subtract,
                                op1=mybir.AluOpType.max)
                        for off, nn_ in chunks:
                            nc.tensor.matmul(
                                ps[32 * s:32 * s + 32, off:off + nn_],
                                w63[c][:, 31 - m:63 - m],
                                rt[:, off:off + nn_],
                                start=(m == 0 and c == 0),
                                stop=(m == 31 and c == 1),
                                tile_position=(0, 32 * s))

                relu = opool.tile([128, W], f32, tag="relu", name="relu")
                o1sb = opool.tile([128, W], f32, tag="o1sb", name="o1sb")
                for s in range(4):
                    r = slice(32 * s, 32 * s + 32)
                    nc.scalar.activation(
                        out=relu[r, :], in_=ps[r, :],
                        func=mybir.ActivationFunctionType.Relu,
                        bias=sib[r, 0:1], scale=1.0)
                    nc.vector.tensor_tensor(
                        out=o1sb[r, :], in0=relu[r, :], in1=mksb[r, :],
                        op=mybir.AluOpType.mult)
                    nc.sync.dma_start(out=o1_d[r, :], in_=o1sb[r, :])

    nc.compile()
    return nc



def _build_strip(reps: int, Ws: int):
    """Per-strip window variant: each 32-row PSUM strip s gets its own
    Ws-column window [32s - padS, 32s + 32 + padS), padS = (Ws-32)/2, so
    producer tiles and matmuls are [128, Ws] instead of the 128-row union
    window.  Valid when max group size <= padS + 1."""
    import contextlib

    import concourse.mybir as mybir
    from concourse import bacc, tile

    f32 = mybir.dt.float32
    f16 = mybir.dt.float16

    nc = bacc.Bacc("TRN2", target_bir_lowering=False, debug=False,
                   num_devices=NCORES)

    xt_d = nc.dram_tensor("xt", [2, 4, 128, Ws], f16, kind="ExternalInput")
    xc_d = nc.dram_tensor("xc", [2, 128, RPC], f32, kind="ExternalInput")
    nx_d = nc.dram_tensor("nx", [2, 128, RPC], f32, kind="ExternalInput")
    w63_d = nc.dram_tensor("w63", [2, 128, 63], f16, kind="ExternalInput")
    sib_d = nc.dram_tensor("sib", [128, 1], f32, kind="ExternalInput")
    brow_d = nc.dram_tensor("brow", [128, Ws], f32, kind="ExternalInput")
    bcol_d = nc.dram_tensor("bcol", [128, 1], f32, kind="ExternalInput")
    o1_d = nc.dram_tensor("o1", [128, Ws], f32, kind="ExternalOutput")
    mk_d = nc.dram_tensor("mk", [128, Ws], f32, kind="ExternalOutput")

    with tile.TileContext(nc) as tc:
        with (
            tc.tile_pool(name="const", bufs=1) as cpool,
            tc.tile_pool(name="rhs", bufs=16) as rpool,
            tc.tile_pool(name="out", bufs=1) as opool,
            tc.tile_pool(name="psum", bufs=1, space="PSUM") as ppool,
        ):
            xt = [[cpool.tile([128, Ws], f16, tag=f"xt{c}{st}",
                              name=f"xt{c}{st}") for st in range(4)]
                  for c in range(2)]
            xc = [cpool.tile([128, RPC], f32, tag=f"xc{c}", name=f"xc{c}")
                  for c in range(2)]
            nx = [cpool.tile([128, RPC], f32, tag=f"nx{c}", name=f"nx{c}")
                  for c in range(2)]
            w63 = [cpool.tile([128, 63], f16, tag=f"w63{c}", name=f"w63{c}")
                   for c in range(2)]
            sib = cpool.tile([128, 1], f32, tag="sib", name="sib")
            brow = cpool.tile([128, Ws], f32, tag="brow", name="brow")
            bcol = cpool.tile([128, 1], f32, tag="bcol", name="bcol")

            for c in range(2):
                for st in range(4):
                    nc.sync.dma_start(out=xt[c][st][:], in_=xt_d[c, st])
                nc.sync.dma_start(out=xc[c][:], in_=xc_d[c])
                nc.sync.dma_start(out=nx[c][:], in_=nx_d[c])
                nc.sync.dma_start(out=w63[c][:], in_=w63_d[c])
            nc.sync.dma_start(out=sib[:], in_=sib_d[:])
            nc.sync.dma_start(out=brow[:], in_=brow_d[:])
            nc.sync.dma_start(out=bcol[:], in_=bcol_d[:])

            loop_cm = (tc.For_i(0, reps, 1, staggered_reset=True)
                       if reps > 1 else contextlib.nullcontext())
            with loop_cm:
                mksb = opool.tile([128, Ws], f32, tag="mksb", name="mksb")
                nc.vector.tensor_scalar(
                    out=mksb[:], in0=brow[:], scalar1=bcol[:, 0:1],
                    scalar2=None, op0=mybir.AluOpType.is_equal)
                nc.sync.dma_start(out=mk_d[:], in_=mksb[:])

                ps = ppool.tile([128, Ws], f32, tag="ps", name="ps")
                for q in range(RPC):
                    s, m = q % 4, q // 4
                    on_act = (q % 4 == 3)
                    for c in range(2):
                        rt = rpool.tile([128, Ws], f16, tag="rt", name="rt")
                        if on_act:
                            nc.scalar.activation(
                                out=rt[:], in_=xt[c][s][:],
                                func=mybir.ActivationFunctionType.Relu,
                                bias=nx[c][:, q:q + 1], scale=1.0)
                        else:
                            nc.vector.tensor_scalar(
                                out=rt[:], in0=xt[c][s][:],
                                scalar1=xc[c][:, q:q + 1], scalar2=0.0,
                                op0=mybir.AluOpType.subtract,
                                op1=mybir.AluOpType.max)
                        for off, nn_ in _nchunks(Ws):
                            nc.tensor.matmul(
                                ps[32 * s:32 * s + 32, off:off + nn_],
                                w63[c][:, 31 - m:63 - m],
                                rt[:, off:off + nn_],
                                start=(m == 0 and c == 0),
                                stop=(m == 31 and c == 1),
                                tile_position=(0, 32 * s))

                relu = opool.tile([128, Ws], f32, tag="relu", name="relu")
                o1sb = opool.tile([128, Ws], f32, tag="o1sb", name="o1sb")
                nc.scalar.activation(
                    out=relu[:], in_=ps[:],
                    func=mybir.ActivationFunctionType.Relu,
                    bias=sib[:, 0:1], scale=1.0)
                nc.vector.tensor_tensor(
                    out=o1sb[:], in0=relu[:], in1=mksb[:],
                    op=mybir.AluOpType.mult)
                nc.sync.dma_start(out=o1_d[:], in_=o1sb[:])

    nc.compile()
    return nc


def _prep_inputs_strip(xs, batch, w, b, Ws):
    xs = np.ascontiguousarray(xs, dtype=np.float32)
    batch_f = np.ascontiguousarray(batch, dtype=np.float32)
    w = np.asarray(w, dtype=np.float32)
    bval = np.float32(np.asarray(b).reshape(-1)[0])
    padS = (Ws - 32) // 2

    xsT = xs.T
    xsT16 = xsT.astype(np.float16)
    s_vec = (xs @ w + bval).astype(np.float32)

    w63 = np.zeros((2, 128, 63), np.float16)
    w63[0, :, 31] = w[:128].astype(np.float16)
    w63[1, :, 31] = w[128:].astype(np.float16)

    perm = np.array(_P_OF_Q)
    ar = np.arange(Ws)

    in_maps = []
    for cid in range(NCORES):
        r0 = cid * RPC
        xblk = xsT[:, r0:r0 + RPC]
        xp = xblk[:, perm]
        # per-strip window columns and inputs
        xt = np.empty((2, 4, 128, Ws), np.float16)
        brow = np.empty((128, Ws), np.float32)
        for st in range(4):
            cols = (r0 + 32 * st - padS + ar) % N
            xt[:, st] = xsT16[:, cols].reshape(2, 128, Ws)
            brow[32 * st:32 * st + 32] = batch_f[cols]
        # poison diagonal: row p (strip p//32) has its own column at
        # local index (p % 32) + padS
        brow[np.arange(128), (np.arange(128) % 32) + padS] += 0.5
        in_maps.append({
            "xt": xt,
            "xc": np.ascontiguousarray(xp.reshape(2, 128, RPC)),
            "nx": np.ascontiguousarray((-xp).reshape(2, 128, RPC)),
            "w63": w63,
            "sib": np.ascontiguousarray(s_vec[r0:r0 + RPC, None]),
            "brow": brow,
            "bcol": np.ascontiguousarray(batch_f[r0:r0 + RPC, None]),
        })
    return in_maps


def _assemble_strip(res, Ws):
    padS = (Ws - 32) // 2
    out1 = np.zeros((N, N), np.float32)
    mask = np.zeros((N, N), np.float32)
    ar = np.arange(Ws)
    for cid in range(NCORES):
        r0 = cid * RPC
        for st in range(4):
            cols = (r0 + 32 * st - padS + ar) % N
            rr = slice(r0 + 32 * st, r0 + 32 * st + 32)
            out1[rr, cols] = res["o1"][cid][32 * st:32 * st + 32]
            mask[rr, cols] = res["mk"][cid][32 * st:32 * st + 32]
    return out1, mask


def _pick_W(batch):
    b = np.asarray(batch).astype(np.int64)
    counts = np.bincount(b, minlength=1)
    span = int(counts.max())  # max group size; need pad >= span - 1
    for padS in (48, 64, 80, 112):
        if span - 1 <= padS:
            return ("S", 32 + 2 * padS)
    for Wc in _ALLOWED_W:
        pad = (Wc - RPC) // 2
        if Wc >= N + RPC or pad >= span - 1:
            return Wc
    return _ALLOWED_W[-1]


def _prep_inputs(xs, batch, w, b, W):
    """Host-side slicing/packing/rolling. All O(n*d) numpy."""
    if isinstance(W, tuple):
        return _prep_inputs_strip(xs, batch, w, b, W[1])
    xs = np.ascontiguousarray(xs, dtype=np.float32)
    batch_f = np.ascontiguousarray(batch, dtype=np.float32)
    w = np.asarray(w, dtype=np.float32)
    bval = np.float32(np.asarray(b).reshape(-1)[0])
    pad = (W - RPC) // 2

    xsT = xs.T  # [256, 1024]
    xsT16 = xsT.astype(np.float16)
    s = (xs @ w + bval).astype(np.float32)  # s_i + b

    w63 = np.zeros((2, 128, 63), np.float16)
    w63[0, :, 31] = w[:128].astype(np.float16)
    w63[1, :, 31] = w[128:].astype(np.float16)

    perm = np.array(_P_OF_Q)  # producer col q -> local row perm[q]

    in_maps = []
    for cid in range(NCORES):
        r0 = cid * RPC
        cols = (r0 - pad + np.arange(W)) % N
        xblk = xsT[:, r0:r0 + RPC]          # [256, 128] f32, local rows
        browc = np.ascontiguousarray(
            np.broadcast_to(batch_f[cols], (128, W)).copy())
        # poison the diagonal so is_equal gives 0 there (mask removes i==j)
        browc[np.arange(128), np.arange(128) + pad] += 0.5
        xp = xblk[:, perm]                   # producer order
        in_maps.append({
            "xt": np.ascontiguousarray(xsT16[:, cols].reshape(2, 128, W)),
            "xc": np.ascontiguousarray(xp.reshape(2, 128, RPC)),
            "nx": np.ascontiguousarray((-xp).reshape(2, 128, RPC)),
            "w63": w63,
            "sib": np.ascontiguousarray(s[r0:r0 + RPC, None]),
            "brow": browc,
            "bcol": np.ascontiguousarray(batch_f[r0:r0 + RPC, None]),
        })
    return in_maps


def _get_runner(reps: int, W: int):
    key = ("runner", reps, W)
    if key in _CACHE:
        return _CACHE[key]

    import jax
    import concourse.mybir as mybir
    from concourse.bass2jax import (_bass_exec_p, install_neuronx_cc_hook,
                                    partition_id_tensor)
    from jax.experimental.shard_map import shard_map
    from jax.sharding import Mesh, NamedSharding, PartitionSpec

    nc = (_build_strip(reps, W[1]) if isinstance(W, tuple)
          else _build(reps, W))
    install_neuronx_cc_hook()

    partition_name = (nc.partition_id_tensor.name
                      if nc.partition_id_tensor else None)
    in_names, out_names, out_avals = [], [], []
    for alloc in nc.m.functions[0].allocations:
        if not isinstance(alloc, mybir.MemoryLocationSet):
            continue
        name = alloc.memorylocations[0].name
        if alloc.kind == "ExternalInput":
            if name != partition_name:
                in_names.append(name)
        elif alloc.kind == "ExternalOutput":
            out_names.append(name)
            out_avals.append(jax.core.ShapedArray(
                tuple(alloc.tensor_shape), mybir.dt.np(alloc.dtype)))
    n_params = len(in_names)
    n_outs = len(out_avals)
    in_names_all = in_names + out_names + (
        [partition_name] if partition_name else [])

    def _body(*args):
        operands = list(args)
        if partition_name is not None:
            operands.append(partition_id_tensor())
        return tuple(_bass_exec_p.bind(
            *operands, out_avals=tuple(out_avals),
            in_names=tuple(in_names_all), out_names=tuple(out_names),
            lowering_input_output_aliases=(), sim_require_finite=True,
            sim_require_nnan=True, nc=nc))

    devices = jax.devices()[:NCORES]
    mesh = Mesh(np.asarray(devices), ("core",))
    sharded = jax.jit(shard_map(
        _body, mesh=mesh,
        in_specs=(PartitionSpec("core"),) * (n_params + n_outs),
        out_specs=(PartitionSpec("core"),) * n_outs, check_rep=False))
    sharding = NamedSharding(mesh, PartitionSpec("core"))
    zeros_dev = [jax.device_put(
        np.zeros((NCORES * av.shape[0], *av.shape[1:]), av.dtype), sharding)
        for av in out_avals]

    def upload(in_maps):
        concat_in = [np.concatenate([in_maps[c][n] for c in range(NCORES)],
                                    axis=0) for n in in_names]
        return [jax.device_put(a, sharding) for a in concat_in]

    def execute(dev_in):
        return sharded(*dev_in, *zeros_dev)

    def fetch(outs):
        res = {}
        for i, name in enumerate(out_names):
            arr = np.asarray(outs[i])
            res[name] = arr.reshape(NCORES, *out_avals[i].shape)
        return res

    def run(in_maps):
        return fetch(execute(upload(in_maps)))

    run.upload = upload
    run.execute = execute
    run.fetch = fetch
    _CACHE[key] = run
    return run


def _assemble(res, W):
    if isinstance(W, tuple):
        return _assemble_strip(res, W[1])
    pad = (W - RPC) // 2
    out1 = np.zeros((N, N), np.float32)
    mask = np.zeros((N, N), np.float32)
    ar = np.arange(W)
    for cid in range(NCORES):
        r0 = cid * RPC
        cols = (r0 - pad + ar) % N
        out1[r0:r0 + RPC, cols] = res["o1"][cid]
        mask[r0:r0 + RPC, cols] = res["mk"][cid]
    return out1, mask


def kernel(xs, batch, w, b):
    W = _pick_W(batch)
    run = _get_runner(1, W)
    in_maps = _prep_inputs(xs, batch, w, b, W)
    res = run(in_maps)
    out1, mask = _assemble(res, W)
    counts = np.bincount(np.asarray(batch).astype(np.int64), minlength=1)
    count = np.float32((counts.astype(np.int64) ** 2).sum() - N)
    return out1, mask, count


if __name__ == "__main__":
    rng = np.random.default_rng(0)
    xs = rng.standard_normal((N, D), dtype=np.float32)
    batch = np.sort(rng.integers(0, 32, N)).astype(np.int64)
    w = (rng.standard_normal(D, dtype=np.float32) / 16.0)
    b = rng.standard_normal(1, dtype=np.float32)
    t0 = time.time()
    o1, mk, cnt = kernel(xs=xs, batch=batch, w=w, b=b)
    print("kernel ran in", time.time() - t0, "W =", _pick_W(batch))
    pm = np.maximum(xs[:, None, :], xs[None, :, :])
    dp = pm @ w + b[0]
    same = (batch[:, None] == batch[None, :]).astype(np.float32)
    m_ref = same * (1.0 - np.eye(N, dtype=np.float32))
    o_ref = np.maximum(dp * m_ref, 0.0)
    print("out1 err:", np.abs(o1 - o_ref).max(),
          "rel:", np.abs(o1 - o_ref).max() / np.abs(o_ref).max())
    print("mask err:", np.abs(mk - m_ref).max())
    print("count:", cnt, "ref:", m_ref.sum())
